# revision 1
# baseline (speedup 1.0000x reference)
"""Two-layer dropout-masked LSTM (B=512, T=256, I=64, H0=128, H1=32) on 8 trn2 cores.

Data-parallel over batch: 64 rows/core. State kept transposed [feature, batch].
Per-step PSUM bank layout (512 f32 cols): [i0|f0|o0|g0'|i1|f1|o1|g1'] where the
L1 block holds the *previous* step's layer-1 gates (L1 runs one iteration behind
L0 so sigmoids cover whole banks). tanh(g) = 2*sigmoid(2g) - 1 with the 2x
pre-scaled into the g-gate weights, so ACT only runs Sigmoid + one Tanh(c) per
step. Fused scalar_tensor_tensor keeps DVE at ~7 ops/step.

The toolchain's walrus build supports at most ONE semaphore wait per
instruction, so the program is structured to never need two: all static data
(weights/bias/x/masks) arrives via five upfront DMAs, masks live in SBUF as
uint8 {0,1} with the dropout scale folded into fused scalar_tensor_tensor ops
(no per-step DMAs at all), state inits run on DVE, an ACT preamble absorbs the
const-DMA tick + loads the sigmoid/tanh table set, and a tiny per-group PE
dummy matmul absorbs the PSUM-slot WAR tick.
"""

import numpy as np

B, T, I, H0, H1 = 512, 256, 64, 128, 32
NCORES = 8
BS = B // NCORES  # 64
# col-block order within a PSUM step-bank: i, f, o, g (pytorch rows are i,f,g,o)
GATE_ORDER = [0, 1, 3, 2]
G_GATE = 2  # pytorch block index of the tanh gate, pre-scaled by 2

# packed-constant tensor column layout
C_WIH0 = 0        # rows 0:65,  cols 0:512
C_WHH0 = 512      # rows 0:128, cols 512:1024
C_WIH1 = 1024     # rows 0:128, cols 1024:1152
C_WHH1 = 1152     # rows 0:33,  cols 1152:1280
C_WLIN = 1280     # rows 0:33,  col 1280
C_BIAS = 1281     # zeros col
C_COLS = 1282
MASK_SCALE = float(np.float32(1.0) / np.float32(1.0 - 0.4))

_CACHED = {}


def _build_program(debug_steps=(), n_steps=T):
    import os
    import concourse.bass as bass
    import concourse.tile as tile
    from concourse import mybir
    from contextlib import ExitStack

    ABL = set(os.environ.get("K_ABLATE", "").split(","))

    f32 = mybir.dt.float32
    u8 = mybir.dt.uint8
    AF = mybir.ActivationFunctionType
    ALU = mybir.AluOpType

    nc = bass.Bass()

    MCOLS = (T + 1) * 128
    cst_d = nc.declare_dram_parameter("cst", [128, C_COLS], f32, isOutput=False)
    xt_d = nc.declare_dram_parameter("xt", [I + 1, T * BS], f32, isOutput=False)
    mph_d = nc.declare_dram_parameter("mph", [128, MCOLS], u8, isOutput=False)
    mpc_d = nc.declare_dram_parameter("mpc", [128, MCOLS], u8, isOutput=False)
    y_d = nc.declare_dram_parameter("y", [BS, 1], f32, isOutput=True)
    dbg_d = {}
    for dt_ in debug_steps:
        dbg_d[dt_] = {
            name: nc.declare_dram_parameter(f"dbg_{name}_{dt_}", shape, f32, isOutput=True)
            for name, shape in (
                ("S", [128, 512]), ("vc", [128, 128]), ("c", [128, 128]),
                ("T", [128, 128]), ("o2", [128, 128]), ("h0", [H0, BS]),
                ("h1", [H1 + 1, BS]), ("bank", [128, 512]),
            )
        }

    GRP = 4  # timesteps per PSUM tile (4 banks); bufs=2 -> all 8 banks

    with ExitStack() as ctx:
        tc = ctx.enter_context(tile.TileContext(nc))
        const = ctx.enter_context(tc.tile_pool(name="const", bufs=1))
        xpool = ctx.enter_context(tc.tile_pool(name="xtp", bufs=1))
        psum = ctx.enter_context(
            tc.tile_pool(name="gates", bufs=2, space=bass.MemorySpace.PSUM)
        )
        spool = ctx.enter_context(tc.tile_pool(name="sig", bufs=2))
        mpool = ctx.enter_context(tc.tile_pool(name="masks", bufs=1))
        wpool = ctx.enter_context(tc.tile_pool(name="work", bufs=2))
        hpool = ctx.enter_context(tc.tile_pool(name="state", bufs=2))

        cst = const.tile([128, C_COLS], f32)
        nc.sync.dma_start(cst[:], cst_d[:])
        xt = xpool.tile([I + 1, T * BS], f32)
        nc.sync.dma_start(xt[:], xt_d[:])
        mph = mpool.tile([128, MCOLS], u8, tag="mph")
        nc.sync.dma_start(mph[:], mph_d[:])
        mpc = mpool.tile([128, MCOLS], u8, tag="mpc")
        nc.sync.dma_start(mpc[:], mpc_d[:])

        bias0 = cst[:, C_BIAS : C_BIAS + 1]

        # ACT preamble: absorb the cst DMA tick on ACT and preload the
        # sigmoid/tanh table set before the timestep loop.
        scratch = const.tile([128, 1], f32)
        nc.scalar.activation(scratch[:], cst[:, 0:1], AF.Copy)
        nc.scalar.activation(scratch[:], cst[:, 0:1], AF.Sigmoid, bias=bias0)
        nc.scalar.activation(scratch[:], cst[:, 0:1], AF.Tanh, bias=bias0)
        # DVE preamble: absorb the two mask DMA ticks so in-loop mask reads
        # never add a second wait on top of same-engine pipeline waits.
        scr8 = const.tile([1, 2], u8)
        nc.vector.tensor_copy(scr8[0:1, 0:1], mph[0:1, 0:1])
        nc.vector.tensor_copy(scr8[0:1, 1:2], mpc[0:1, 0:1])

        # ---- initial state (DVE so consumers' waits stay single-source) ----
        h0_prev = hpool.tile([H0, BS], f32, tag="h0")
        nc.vector.memset(h0_prev[:], 0.0)
        h1_slot0 = hpool.tile([H1 + 1, BS], f32, tag="h1")
        nc.vector.memset(h1_slot0[H1 : H1 + 1, :], 1.0)
        h1_prev = hpool.tile([H1 + 1, BS], f32, tag="h1")
        nc.vector.memset(h1_prev[0:H1, :], 0.0)
        nc.vector.memset(h1_prev[H1 : H1 + 1, :], 1.0)
        c_prev = wpool.tile([128, 128], f32, tag="c")
        nc.vector.memset(c_prev[:], 0.0)

        pt = None
        ptv = None
        for t in range(n_steps + 1):
            k, s = divmod(t, GRP)
            if s == 0:
                pt = psum.tile([128, GRP * 512], f32, tag="gates")
                ptv = pt[:].rearrange("p (s c) -> p s c", s=GRP)
                if "dummy" not in ABL:
                    # tiny dummy matmul: absorbs the PSUM-slot WAR (ACT sigma
                    # readers from group k-2) onto PE before any real writer.
                    nc.tensor.matmul(
                        ptv[0:1, GRP - 1, 256:257],
                        cst[0:1, 0:1],
                        cst[0:1, 0:1],
                        start=True,
                        stop=True,
                    )

            if t < n_steps and "rec" not in ABL:
                # L0 gates for step t: x-part (bias via ones row) + recurrent,
                # as immediately-paired accumulation groups (interleaving
                # start=True groups with deferred start=False continuations
                # corrupts PSUM on this toolchain).
                for j in range(4):
                    nc.tensor.matmul(
                        ptv[:, s, j * 64 : (j + 1) * 64],
                        cst[0 : I + 1, C_WIH0 + j * 128 : C_WIH0 + (j + 1) * 128],
                        xt[:, t * BS : (t + 1) * BS],
                        start=True,
                        stop=False,
                    )
                    nc.tensor.matmul(
                        ptv[:, s, j * 64 : (j + 1) * 64],
                        cst[0:H0, C_WHH0 + j * 128 : C_WHH0 + (j + 1) * 128],
                        h0_prev[:],
                        start=False,
                        stop=True,
                    )
            if t >= 1 and "l1" not in ABL:
                # L1 gates for step t-1 (uses h0_{t-1}, h1_{t-2}; bias via ones row)
                for j in range(4):
                    nc.tensor.matmul(
                        ptv[0:H1, s, 256 + j * 64 : 256 + (j + 1) * 64],
                        cst[0:H0, C_WIH1 + j * H1 : C_WIH1 + (j + 1) * H1],
                        h0_prev[:],
                        start=True,
                        stop=False,
                    )
                    nc.tensor.matmul(
                        ptv[0:H1, s, 256 + j * 64 : 256 + (j + 1) * 64],
                        cst[0 : H1 + 1, C_WHH1 + j * H1 : C_WHH1 + (j + 1) * H1],
                        h1_prev[:],
                        start=False,
                        stop=True,
                    )

            # ---- sigmoids over the whole bank ----
            S = spool.tile([128, 512], f32, tag="S")
            if t < n_steps:
                nc.scalar.activation(
                    S[:, 0:256], ptv[:, s, 0:256], AF.Sigmoid, bias=bias0
                )
            if t >= 1:
                nc.scalar.activation(
                    S[0:H1, 256:512],
                    ptv[0:H1, s, 256:512],
                    AF.Sigmoid,
                    bias=bias0[0:H1, :],
                )

            # 3D views: [128, 2 blocks, 64]; block 0 = L0 step t, block 1 = L1 step t-1
            Sv = S[:].rearrange("p (a c) -> p a c", a=2)
            si = Sv[:, :, 0:64]
            sf = Sv[:, :, 64:128]
            so = Sv[:, :, 128:192]
            sg = Sv[:, :, 192:256]

            Pp = wpool.tile([128, 128], f32, tag="Pp")
            Ppv = Pp[:].rearrange("p (a c) -> p a c", a=2)
            v = wpool.tile([128, 128], f32, tag="v")
            vv = v[:].rearrange("p (a c) -> p a c", a=2)
            vc = wpool.tile([128, 128], f32, tag="vc")
            vcv = vc[:].rearrange("p (a c) -> p a c", a=2)
            c_new = wpool.tile([128, 128], f32, tag="c")
            cnv = c_new[:].rearrange("p (a c) -> p a c", a=2)
            cpv = c_prev[:].rearrange("p (a c) -> p a c", a=2)
            Mcv = mpc[:, t * 128 : (t + 1) * 128].rearrange("p (a c) -> p a c", a=2)
            Mhv = mph[:, t * 128 : (t + 1) * 128].rearrange("p (a c) -> p a c", a=2)

            # P' = (sig(g') - 0.5) * sig(i)   [= tanh(g)*sig(i)/2]
            nc.vector.scalar_tensor_tensor(Ppv, sg, 0.5, si, ALU.subtract, ALU.mult)
            # v = sig(f) * c_prev
            nc.vector.tensor_tensor(vv, sf, cpv, ALU.mult)
            # vc = 2*P' + v
            nc.vector.scalar_tensor_tensor(vcv, Ppv, 2.0, vv, ALU.mult, ALU.add)
            # c = (vc * dropout_scale) * mask_c_u8
            nc.vector.scalar_tensor_tensor(cnv, vcv, MASK_SCALE, Mcv, ALU.mult, ALU.mult)
            if t == 0:
                # layer-1 half of the c state must start at zero (kills psum junk)
                nc.vector.memset(c_new[:, 64:128], 0.0)

            Tt = wpool.tile([128, 128], f32, tag="T")
            # h uses tanh of the UNMASKED cell state (mask only hits the carry)
            nc.scalar.activation(Tt[:], vc[:], AF.Tanh, bias=bias0)

            o2 = wpool.tile([128, 128], f32, tag="o2")
            o2v = o2[:].rearrange("p (a c) -> p a c", a=2)
            # o'' = (sig(o) * dropout_scale) * mask_h_u8
            nc.vector.scalar_tensor_tensor(o2v, so, MASK_SCALE, Mhv, ALU.mult, ALU.mult)

            if t < n_steps:
                h0_new = hpool.tile([H0, BS], f32, tag="h0")
                nc.vector.tensor_tensor(h0_new[:], o2[:, 0:64], Tt[:, 0:64], ALU.mult)
                h0_prev = h0_new
            if t >= 1:
                h1_new = hpool.tile([H1 + 1, BS], f32, tag="h1")
                nc.vector.tensor_tensor(
                    h1_new[0:H1, :], o2[0:H1, 64:128], Tt[0:H1, 64:128], ALU.mult
                )
                h1_prev = h1_new
            c_prev = c_new
            if t in dbg_d:
                dd = dbg_d[t]
                bank_sb = spool.tile([128, 512], f32, tag="bankdbg")
                nc.scalar.copy(bank_sb[:], ptv[:, s, :])
                nc.sync.dma_start(dd["bank"][:], bank_sb[:])
                nc.sync.dma_start(dd["S"][:], S[:])
                nc.sync.dma_start(dd["vc"][:], vc[:])
                nc.sync.dma_start(dd["c"][:], c_new[:])
                nc.sync.dma_start(dd["T"][:], Tt[:])
                nc.sync.dma_start(dd["o2"][:], o2[:])
                nc.sync.dma_start(dd["h0"][:], h0_prev[:])
                nc.sync.dma_start(dd["h1"][:], h1_prev[:])

        # ---- final projection: y = h1_255 @ W_lin.T + b_lin  -> [64, 1] ----
        yp = ptv[0:BS, 1, 0:1]
        nc.tensor.matmul(
            yp, h1_prev[:], cst[0 : H1 + 1, C_WLIN : C_WLIN + 1], start=True, stop=True
        )
        y_sb = const.tile([BS, 1], f32)
        nc.scalar.copy(y_sb[:], yp)
        nc.sync.dma_start(y_d[:], y_sb[:])

    _split_multiwaits(nc)
    return nc


def _split_multiwaits(nc):
    """This toolchain's walrus accepts at most one semaphore wait per
    instruction. Split any extra waits onto standalone EventSemaphore
    instructions inserted just before the offending instruction on the same
    engine queue (in-order execution preserves semantics exactly)."""
    from concourse import mybir

    n = 0
    for fn in nc.m.functions:
        for bb in fn.blocks:
            out = []
            for inst in bb.instructions:
                si = inst.sync_info
                if si is not None and si.on_wait and len(si.on_wait) > 1:
                    waits = list(si.on_wait)
                    for w in waits[:-1]:
                        n += 1
                        out.append(
                            mybir.InstEventSemaphore(
                                name=f"I-wsplit-{n}",
                                engine=inst.engine,
                                ins=[],
                                outs=[],
                                sync_info=mybir.SyncInfo(on_wait=[w], on_update=[]),
                            )
                        )
                    inst.sync_info = mybir.SyncInfo(
                        on_wait=[waits[-1]], on_update=list(si.on_update)
                    )
                out.append(inst)
            bb.instructions = out


def _prep_consts(W_ih0, W_hh0, b_ih0, b_hh0, W_ih1, W_hh1, b_ih1, b_hh1, W_lin, b_lin):
    f = np.float32
    b0 = (b_ih0 + b_hh0).astype(f)
    b1 = (b_ih1 + b_hh1).astype(f)
    cst = np.zeros((128, C_COLS), f)
    for j, g in enumerate(GATE_ORDER):
        m = 2.0 if g == G_GATE else 1.0
        cst[0:I, C_WIH0 + j * H0 : C_WIH0 + (j + 1) * H0] = (
            m * W_ih0[g * H0 : (g + 1) * H0].T
        )
        cst[I, C_WIH0 + j * H0 : C_WIH0 + (j + 1) * H0] = m * b0[g * H0 : (g + 1) * H0]
        cst[0:H0, C_WHH0 + j * H0 : C_WHH0 + (j + 1) * H0] = (
            m * W_hh0[g * H0 : (g + 1) * H0].T
        )
        cst[0:H0, C_WIH1 + j * H1 : C_WIH1 + (j + 1) * H1] = (
            m * W_ih1[g * H1 : (g + 1) * H1].T
        )
        cst[0:H1, C_WHH1 + j * H1 : C_WHH1 + (j + 1) * H1] = (
            m * W_hh1[g * H1 : (g + 1) * H1].T
        )
        cst[H1, C_WHH1 + j * H1 : C_WHH1 + (j + 1) * H1] = m * b1[g * H1 : (g + 1) * H1]
    cst[0:H1, C_WLIN] = W_lin[0]
    cst[H1, C_WLIN] = b_lin[0]
    return cst


def kernel(
    input_seq,
    mask_h0,
    mask_c0,
    mask_h1,
    mask_c1,
    W_ih0,
    W_hh0,
    b_ih0,
    b_hh0,
    W_ih1,
    W_hh1,
    b_ih1,
    b_hh1,
    W_lin,
    b_lin,
):
    import sys

    for p in ("/opt/trn_rl_repo",):
        if p not in sys.path:
            sys.path.insert(0, p)
    from concourse.bass_utils import run_bass_kernel_spmd

    f = np.float32
    input_seq = np.asarray(input_seq, f)
    mask_h0 = np.asarray(mask_h0, f)
    mask_c0 = np.asarray(mask_c0, f)
    mask_h1 = np.asarray(mask_h1, f)
    mask_c1 = np.asarray(mask_c1, f)
    args = [np.asarray(a, f) for a in (W_ih0, W_hh0, b_ih0, b_hh0,
                                       W_ih1, W_hh1, b_ih1, b_hh1, W_lin, b_lin)]
    cst = _prep_consts(*args)

    in_maps = []
    for c in range(NCORES):
        lo, hi = c * BS, (c + 1) * BS
        xs = input_seq[lo:hi]  # [BS, T, I]
        xt = np.empty((I + 1, T * BS), f)
        xt[0:I] = xs.transpose(2, 1, 0).reshape(I, T * BS)
        xt[I] = 1.0
        u8 = np.uint8
        mph3 = np.zeros((T + 1, 128, 128), u8)
        mpc3 = np.zeros((T + 1, 128, 128), u8)
        mph3[0:T, :, 0:64] = (mask_h0[:, lo:hi, :] != 0).transpose(0, 2, 1)
        mph3[1 : T + 1, 0:H1, 64:128] = (mask_h1[:, lo:hi, :] != 0).transpose(0, 2, 1)
        mpc3[0:T, :, 0:64] = (mask_c0[:, lo:hi, :] != 0).transpose(0, 2, 1)
        mpc3[1 : T + 1, 0:H1, 64:128] = (mask_c1[:, lo:hi, :] != 0).transpose(0, 2, 1)
        mph = np.ascontiguousarray(mph3.transpose(1, 0, 2).reshape(128, -1))
        mpc = np.ascontiguousarray(mpc3.transpose(1, 0, 2).reshape(128, -1))
        in_maps.append({"cst": cst, "xt": xt, "mph": mph, "mpc": mpc})

    if "nc" not in _CACHED:
        _CACHED["nc"] = _build_program()
    _CACHED["in_maps"] = in_maps
    res = run_bass_kernel_spmd(_CACHED["nc"], in_maps, list(range(NCORES)))
    out = np.concatenate([res.results[c]["y"] for c in range(NCORES)], axis=0)
    return out.astype(f)



# revision 12
# speedup vs baseline: 1.8948x; 1.8948x over previous
"""Two-layer dropout-masked LSTM (B=512, T=256, I=64, H0=128, H1=32) on 8 trn2 cores.

Data-parallel over batch: 64 rows/core. State kept transposed [feature, batch].
Per-step PSUM bank layout (512 f32 cols): [i0|f0|o0|g0'|i1|f1|o1|g1'] where the
L1 block holds the *previous* step's layer-1 gates (L1 runs one iteration behind
L0 so sigmoids cover whole banks). tanh(g) = 2*sigmoid(2g) - 1 with the 2x
pre-scaled into the g-gate weights, so ACT only runs Sigmoid + one Tanh(c) per
step. Fused scalar_tensor_tensor keeps DVE at ~7 ops/step.

The toolchain's walrus build supports at most ONE semaphore wait per
instruction, so the program is structured to never need two: all static data
(weights/bias/x/masks) arrives via five upfront DMAs, masks live in SBUF as
uint8 {0,1} with the dropout scale folded into fused scalar_tensor_tensor ops
(no per-step DMAs at all), state inits run on DVE, an ACT preamble absorbs the
const-DMA tick + loads the sigmoid/tanh table set, and a tiny per-group PE
dummy matmul absorbs the PSUM-slot WAR tick.
"""

import numpy as np

B, T, I, H0, H1 = 512, 256, 64, 128, 32
NCORES = 8
BS = B // NCORES  # 64
# col-block order within a PSUM step-bank: i, f, o, g (pytorch rows are i,f,g,o)
GATE_ORDER = [0, 1, 3, 2]
G_GATE = 2  # pytorch block index of the tanh gate, pre-scaled by 2

# packed-constant tensor column layout
C_WIH0 = 0        # rows 0:65,  cols 0:512
C_WHH0 = 512      # rows 0:128, cols 512:1024
C_WIH1 = 1024     # rows 0:128, cols 1024:1152
C_WHH1 = 1152     # rows 0:33,  cols 1152:1280
C_WLIN = 1280     # rows 0:33,  col 1280
C_BIAS = 1281     # zeros col
C_COLS = 1282
MASK_SCALE = float(np.float32(1.0) / np.float32(1.0 - 0.4))

_CACHED = {}


def _build_program(debug_steps=(), n_steps=T):
    import os
    import concourse.bass as bass
    import concourse.tile as tile
    from concourse import mybir
    from contextlib import ExitStack

    ABL = set(os.environ.get("K_ABLATE", "").split(","))

    f32 = mybir.dt.float32
    bf16 = mybir.dt.bfloat16
    u8 = mybir.dt.uint8
    AF = mybir.ActivationFunctionType
    ALU = mybir.AluOpType

    nc = bass.Bass()

    MCOLS = (T + 1) * 128
    cst_d = nc.declare_dram_parameter("cst", [128, C_COLS], bf16, isOutput=False)
    xt_d = nc.declare_dram_parameter("xt", [I + 1, T * BS], bf16, isOutput=False)
    mph_d = nc.declare_dram_parameter("mph", [128, MCOLS], u8, isOutput=False)
    mpc_d = nc.declare_dram_parameter("mpc", [128, MCOLS], u8, isOutput=False)
    y_d = nc.declare_dram_parameter("y", [BS, 1], f32, isOutput=True)
    dbg_d = {}
    for dt_ in debug_steps:
        dbg_d[dt_] = {
            name: nc.declare_dram_parameter(f"dbg_{name}_{dt_}", shape, f32, isOutput=True)
            for name, shape in (
                ("S", [128, 512]), ("vc", [128, 128]), ("c", [128, 128]),
                ("T", [128, 128]), ("o2", [128, 128]), ("h0", [H0, BS]),
                ("h1", [H1 + 1, BS]), ("bank", [128, 512]),
            )
        }

    GRP = 4  # timesteps per PSUM tile (4 banks); bufs=2 -> all 8 banks

    with ExitStack() as ctx:
        tc = ctx.enter_context(tile.TileContext(nc))
        const = ctx.enter_context(tc.tile_pool(name="const", bufs=1))
        xpool = ctx.enter_context(tc.tile_pool(name="xtp", bufs=1))
        psum = ctx.enter_context(
            tc.tile_pool(name="gates", bufs=2, space=bass.MemorySpace.PSUM)
        )
        spool = ctx.enter_context(tc.tile_pool(name="sig", bufs=2))
        mpool = ctx.enter_context(tc.tile_pool(name="masks", bufs=1))
        wpool = ctx.enter_context(tc.tile_pool(name="work", bufs=2))
        hpool = ctx.enter_context(tc.tile_pool(name="state", bufs=2))

        cst = const.tile([128, C_COLS], bf16)
        nc.sync.dma_start(cst[:], cst_d[:])
        xt = xpool.tile([I + 1, T * BS], bf16)
        nc.sync.dma_start(xt[:], xt_d[:])
        mph = mpool.tile([128, MCOLS], u8, tag="mph")
        nc.sync.dma_start(mph[:], mph_d[:])
        mpc = mpool.tile([128, MCOLS], u8, tag="mpc")
        nc.sync.dma_start(mpc[:], mpc_d[:])

        bias0 = cst[:, C_BIAS : C_BIAS + 1]

        # ACT preamble: absorb the cst DMA tick on ACT and preload the
        # sigmoid/tanh table set before the timestep loop.
        scratch = const.tile([128, 1], bf16)
        nc.scalar.activation(scratch[:], cst[:, 0:1], AF.Copy)
        nc.scalar.activation(scratch[:], cst[:, 0:1], AF.Sigmoid, bias=bias0)
        nc.scalar.activation(scratch[:], cst[:, 0:1], AF.Tanh, bias=bias0)
        # DVE preamble: absorb the two mask DMA ticks so in-loop mask reads
        # never add a second wait on top of same-engine pipeline waits.
        scr8 = const.tile([1, 2], u8)
        nc.vector.tensor_copy(scr8[0:1, 0:1], mph[0:1, 0:1])
        nc.vector.tensor_copy(scr8[0:1, 1:2], mpc[0:1, 0:1])

        # ---- initial state (DVE so consumers' waits stay single-source) ----
        h0_prev = hpool.tile([H0, BS], bf16, tag="h0")
        nc.vector.memset(h0_prev[:], 0.0)
        h1_slot0 = hpool.tile([H1 + 1, BS], bf16, tag="h1")
        nc.vector.memset(h1_slot0[H1 : H1 + 1, :], 1.0)
        h1_prev = hpool.tile([H1 + 1, BS], bf16, tag="h1")
        nc.vector.memset(h1_prev[0:H1, :], 0.0)
        nc.vector.memset(h1_prev[H1 : H1 + 1, :], 1.0)
        c_prev = wpool.tile([128, 128], f32, tag="c")
        nc.vector.memset(c_prev[:], 0.0)

        pt = None
        ptv = None
        for t in range(n_steps + 1):
            k, s = divmod(t, GRP)
            if s == 0:
                pt = psum.tile([128, GRP * 512], f32, tag="gates")
                ptv = pt[:].rearrange("p (s c) -> p s c", s=GRP)
                if "dummy" not in ABL:
                    # tiny dummy matmul: absorbs the PSUM-slot WAR (ACT sigma
                    # readers from group k-2) onto PE before any real writer.
                    nc.tensor.matmul(
                        ptv[0:1, GRP - 1, 256:257],
                        cst[0:1, 0:1],
                        cst[0:1, 0:1],
                        start=True,
                        stop=True,
                    )

            if t < n_steps and "rec" not in ABL:
                # L0 gates for step t: x-part (bias via ones row) + recurrent,
                # as immediately-paired accumulation groups (interleaving
                # start=True groups with deferred start=False continuations
                # corrupts PSUM on this toolchain).
                for j in range(4):
                    nc.tensor.matmul(
                        ptv[:, s, j * 64 : (j + 1) * 64],
                        cst[0 : I + 1, C_WIH0 + j * 128 : C_WIH0 + (j + 1) * 128],
                        xt[:, t * BS : (t + 1) * BS],
                        start=True,
                        stop=False,
                    )
                    nc.tensor.matmul(
                        ptv[:, s, j * 64 : (j + 1) * 64],
                        cst[0:H0, C_WHH0 + j * 128 : C_WHH0 + (j + 1) * 128],
                        h0_prev[:],
                        start=False,
                        stop=True,
                    )
            if t >= 1 and "l1" not in ABL:
                # L1 gates for step t-1 (uses h0_{t-1}, h1_{t-2}; bias via ones row)
                for j in range(4):
                    nc.tensor.matmul(
                        ptv[0:H1, s, 256 + j * 64 : 256 + (j + 1) * 64],
                        cst[0:H0, C_WIH1 + j * H1 : C_WIH1 + (j + 1) * H1],
                        h0_prev[:],
                        start=True,
                        stop=False,
                    )
                    nc.tensor.matmul(
                        ptv[0:H1, s, 256 + j * 64 : 256 + (j + 1) * 64],
                        cst[0 : H1 + 1, C_WHH1 + j * H1 : C_WHH1 + (j + 1) * H1],
                        h1_prev[:],
                        start=False,
                        stop=True,
                    )

            # ---- sigmoids over the whole bank ----
            S = spool.tile([128, 512], bf16, tag="S")
            if t < n_steps:
                nc.scalar.activation(
                    S[:, 0:256], ptv[:, s, 0:256], AF.Sigmoid, bias=bias0
                )
            if t >= 1:
                nc.scalar.activation(
                    S[0:H1, 256:512],
                    ptv[0:H1, s, 256:512],
                    AF.Sigmoid,
                    bias=bias0[0:H1, :],
                )

            # 3D views: [128, 2 blocks, 64]; block 0 = L0 step t, block 1 = L1 step t-1
            Sv = S[:].rearrange("p (a c) -> p a c", a=2)
            si = Sv[:, :, 0:64]
            sf = Sv[:, :, 64:128]
            so = Sv[:, :, 128:192]
            sg = Sv[:, :, 192:256]

            Pp = wpool.tile([128, 128], f32, tag="Pp")
            Ppv = Pp[:].rearrange("p (a c) -> p a c", a=2)
            v = wpool.tile([128, 128], f32, tag="v")
            vv = v[:].rearrange("p (a c) -> p a c", a=2)
            vc = wpool.tile([128, 128], f32, tag="vc")
            vcv = vc[:].rearrange("p (a c) -> p a c", a=2)
            c_new = wpool.tile([128, 128], f32, tag="c")
            cnv = c_new[:].rearrange("p (a c) -> p a c", a=2)
            cpv = c_prev[:].rearrange("p (a c) -> p a c", a=2)
            Mcv = mpc[:, t * 128 : (t + 1) * 128].rearrange("p (a c) -> p a c", a=2)
            Mhv = mph[:, t * 128 : (t + 1) * 128].rearrange("p (a c) -> p a c", a=2)

            # P' = (sig(g') - 0.5) * sig(i)   [= tanh(g)*sig(i)/2]
            nc.vector.scalar_tensor_tensor(Ppv, sg, 0.5, si, ALU.subtract, ALU.mult)
            # v = sig(f) * c_prev
            nc.vector.tensor_tensor(vv, sf, cpv, ALU.mult)
            # vc = 2*P' + v
            nc.vector.scalar_tensor_tensor(vcv, Ppv, 2.0, vv, ALU.mult, ALU.add)
            # c = (vc * dropout_scale) * mask_c_u8
            nc.vector.scalar_tensor_tensor(cnv, vcv, MASK_SCALE, Mcv, ALU.mult, ALU.mult)
            if t == 0:
                # layer-1 half of the c state must start at zero (kills psum junk)
                nc.vector.memset(c_new[:, 64:128], 0.0)

            Tt = wpool.tile([128, 128], bf16, tag="T")
            # h uses tanh of the UNMASKED cell state (mask only hits the carry)
            nc.scalar.activation(Tt[:], vc[:], AF.Tanh, bias=bias0)

            o2 = wpool.tile([128, 128], bf16, tag="o2")
            o2v = o2[:].rearrange("p (a c) -> p a c", a=2)
            # o'' = (sig(o) * dropout_scale) * mask_h_u8
            nc.vector.scalar_tensor_tensor(o2v, so, MASK_SCALE, Mhv, ALU.mult, ALU.mult)

            if t < n_steps:
                h0_new = hpool.tile([H0, BS], bf16, tag="h0")
                nc.vector.tensor_tensor(h0_new[:], o2[:, 0:64], Tt[:, 0:64], ALU.mult)
                h0_prev = h0_new
            if t >= 1:
                h1_new = hpool.tile([H1 + 1, BS], bf16, tag="h1")
                nc.vector.tensor_tensor(
                    h1_new[0:H1, :], o2[0:H1, 64:128], Tt[0:H1, 64:128], ALU.mult
                )
                h1_prev = h1_new
            c_prev = c_new
            if t in dbg_d:
                dd = dbg_d[t]
                bank_sb = spool.tile([128, 512], f32, tag="bankdbg")
                nc.scalar.copy(bank_sb[:], ptv[:, s, :])
                nc.sync.dma_start(dd["bank"][:], bank_sb[:])
                nc.sync.dma_start(dd["S"][:], S[:])
                nc.sync.dma_start(dd["vc"][:], vc[:])
                nc.sync.dma_start(dd["c"][:], c_new[:])
                nc.sync.dma_start(dd["T"][:], Tt[:])
                nc.sync.dma_start(dd["o2"][:], o2[:])
                nc.sync.dma_start(dd["h0"][:], h0_prev[:])
                nc.sync.dma_start(dd["h1"][:], h1_prev[:])

        # ---- final projection: y = h1_255 @ W_lin.T + b_lin  -> [64, 1] ----
        yp = ptv[0:BS, 1, 0:1]
        nc.tensor.matmul(
            yp, h1_prev[:], cst[0 : H1 + 1, C_WLIN : C_WLIN + 1], start=True, stop=True
        )
        y_sb = const.tile([BS, 1], f32)
        nc.scalar.copy(y_sb[:], yp)
        nc.sync.dma_start(y_d[:], y_sb[:])

    _split_multiwaits(nc)
    return nc


def _split_multiwaits(nc):
    """This toolchain's walrus accepts at most one semaphore wait per
    instruction. Split any extra waits onto standalone EventSemaphore
    instructions inserted just before the offending instruction on the same
    engine queue (in-order execution preserves semantics exactly)."""
    from concourse import mybir

    n = 0
    for fn in nc.m.functions:
        for bb in fn.blocks:
            out = []
            for inst in bb.instructions:
                si = inst.sync_info
                if si is not None and si.on_wait and len(si.on_wait) > 1:
                    waits = list(si.on_wait)
                    for w in waits[:-1]:
                        n += 1
                        out.append(
                            mybir.InstEventSemaphore(
                                name=f"I-wsplit-{n}",
                                engine=inst.engine,
                                ins=[],
                                outs=[],
                                sync_info=mybir.SyncInfo(on_wait=[w], on_update=[]),
                            )
                        )
                    inst.sync_info = mybir.SyncInfo(
                        on_wait=[waits[-1]], on_update=list(si.on_update)
                    )
                out.append(inst)
            bb.instructions = out


def _prep_consts(W_ih0, W_hh0, b_ih0, b_hh0, W_ih1, W_hh1, b_ih1, b_hh1, W_lin, b_lin):
    f = np.float32
    import ml_dtypes
    b0 = (b_ih0 + b_hh0).astype(f)
    b1 = (b_ih1 + b_hh1).astype(f)
    cst = np.zeros((128, C_COLS), f)
    for j, g in enumerate(GATE_ORDER):
        m = 2.0 if g == G_GATE else 1.0
        cst[0:I, C_WIH0 + j * H0 : C_WIH0 + (j + 1) * H0] = (
            m * W_ih0[g * H0 : (g + 1) * H0].T
        )
        cst[I, C_WIH0 + j * H0 : C_WIH0 + (j + 1) * H0] = m * b0[g * H0 : (g + 1) * H0]
        cst[0:H0, C_WHH0 + j * H0 : C_WHH0 + (j + 1) * H0] = (
            m * W_hh0[g * H0 : (g + 1) * H0].T
        )
        cst[0:H0, C_WIH1 + j * H1 : C_WIH1 + (j + 1) * H1] = (
            m * W_ih1[g * H1 : (g + 1) * H1].T
        )
        cst[0:H1, C_WHH1 + j * H1 : C_WHH1 + (j + 1) * H1] = (
            m * W_hh1[g * H1 : (g + 1) * H1].T
        )
        cst[H1, C_WHH1 + j * H1 : C_WHH1 + (j + 1) * H1] = m * b1[g * H1 : (g + 1) * H1]
    cst[0:H1, C_WLIN] = W_lin[0]
    cst[H1, C_WLIN] = b_lin[0]
    return cst.astype(ml_dtypes.bfloat16)


def kernel(
    input_seq,
    mask_h0,
    mask_c0,
    mask_h1,
    mask_c1,
    W_ih0,
    W_hh0,
    b_ih0,
    b_hh0,
    W_ih1,
    W_hh1,
    b_ih1,
    b_hh1,
    W_lin,
    b_lin,
):
    import sys

    for p in ("/opt/trn_rl_repo",):
        if p not in sys.path:
            sys.path.insert(0, p)
    from concourse.bass_utils import run_bass_kernel_spmd

    f = np.float32
    input_seq = np.asarray(input_seq, f)
    mask_h0 = np.asarray(mask_h0, f)
    mask_c0 = np.asarray(mask_c0, f)
    mask_h1 = np.asarray(mask_h1, f)
    mask_c1 = np.asarray(mask_c1, f)
    args = [np.asarray(a, f) for a in (W_ih0, W_hh0, b_ih0, b_hh0,
                                       W_ih1, W_hh1, b_ih1, b_hh1, W_lin, b_lin)]
    cst = _prep_consts(*args)

    in_maps = []
    import ml_dtypes

    for c in range(NCORES):
        lo, hi = c * BS, (c + 1) * BS
        xs = input_seq[lo:hi]  # [BS, T, I]
        xt = np.empty((I + 1, T * BS), ml_dtypes.bfloat16)
        xt[0:I] = xs.transpose(2, 1, 0).reshape(I, T * BS)
        xt[I] = 1.0
        u8 = np.uint8
        mph3 = np.zeros((T + 1, 128, 128), u8)
        mpc3 = np.zeros((T + 1, 128, 128), u8)
        mph3[0:T, :, 0:64] = (mask_h0[:, lo:hi, :] != 0).transpose(0, 2, 1)
        mph3[1 : T + 1, 0:H1, 64:128] = (mask_h1[:, lo:hi, :] != 0).transpose(0, 2, 1)
        mpc3[0:T, :, 0:64] = (mask_c0[:, lo:hi, :] != 0).transpose(0, 2, 1)
        mpc3[1 : T + 1, 0:H1, 64:128] = (mask_c1[:, lo:hi, :] != 0).transpose(0, 2, 1)
        mph = np.ascontiguousarray(mph3.transpose(1, 0, 2).reshape(128, -1))
        mpc = np.ascontiguousarray(mpc3.transpose(1, 0, 2).reshape(128, -1))
        in_maps.append({"cst": cst, "xt": xt, "mph": mph, "mpc": mpc})

    if "nc" not in _CACHED:
        _CACHED["nc"] = _build_program()
    _CACHED["in_maps"] = in_maps
    res = run_bass_kernel_spmd(_CACHED["nc"], in_maps, list(range(NCORES)))
    out = np.concatenate([res.results[c]["y"] for c in range(NCORES)], axis=0)
    return out.astype(f)



# revision 13
# speedup vs baseline: 3.1341x; 1.6540x over previous
"""Two-layer dropout-masked LSTM (B=512, T=256, I=64, H0=128, H1=32) on 8 trn2 cores.

Data-parallel over batch: 64 rows/core. State kept transposed [feature, batch],
all matmul operands bf16 (PSUM accumulates f32); c-state math stays f32.

PSUM layout: two long-lived 4-bank tiles, bank = gate (order i,f,o,g).
  TL0 (layer 0): bank j holds 8 step-slots of 64 cols; the x+bias part of 8
  steps is batched into ONE N=512 matmul per gate per round (start=True also
  clears the bank's has_written bits for the round); the per-step recurrent
  matmul (start=False) accumulates into its 64-col slot.
  TL1 (layer 1): bank j holds 8 step-slots; per step one x-part matmul
  (vs h0_t) and one recurrent matmul (vs h1_{t-1}, bias via ones row).

L0 and L1 run as two decoupled pipelines (L1 lags by one step) with separate
sigmoid calls and separate elementwise sets, so only L0's 4 recurrent matmuls
+ sigmoid + P'/v/vc + tanh + h0 sit on the serial recurrence chain; L1's
matmuls and ops execute in the chain's engine-idle shadow.

The toolchain's walrus build supports at most ONE semaphore wait per
instruction: _split_multiwaits() moves extra waits onto standalone
EventSemaphore instructions. An ACT preamble absorbs the const-DMA tick and
preloads the sigmoid/tanh table set; a DVE preamble absorbs the mask DMA
ticks. tanh(g) = 2*sigmoid(2g) - 1 with the 2x pre-scaled into the g-gate
weights.
"""

import numpy as np

B, T, I, H0, H1 = 512, 256, 64, 128, 32
NCORES = 8
BS = B // NCORES  # 64
# col-block order within the per-gate banks: i, f, o, g (pytorch rows i,f,g,o)
GATE_ORDER = [0, 1, 3, 2]
G_GATE = 2  # pytorch block index of the tanh gate, pre-scaled by 2

# packed-constant tensor column layout
C_WIH0 = 0        # rows 0:65,  cols 0:512
C_WHH0 = 512      # rows 0:128, cols 512:1024
C_WIH1 = 1024     # rows 0:128, cols 1024:1152
C_WHH1 = 1152     # rows 0:33,  cols 1152:1280
C_WLIN = 1280     # rows 0:33,  col 1280
C_BIAS = 1281     # zeros col
C_COLS = 1282
MASK_SCALE = float(np.float32(1.0) / np.float32(1.0 - 0.4))

_CACHED = {}


def _build_program(n_steps=T):
    import concourse.bass as bass
    import concourse.tile as tile
    from concourse import mybir
    from contextlib import ExitStack

    f32 = mybir.dt.float32
    bf16 = mybir.dt.bfloat16
    u8 = mybir.dt.uint8
    AF = mybir.ActivationFunctionType
    ALU = mybir.AluOpType

    nc = bass.Bass()

    MCOLS = (T + 1) * 128
    cst_d = nc.declare_dram_parameter("cst", [128, C_COLS], bf16, isOutput=False)
    xt_d = nc.declare_dram_parameter("xt", [I + 1, T * BS], bf16, isOutput=False)
    mph_d = nc.declare_dram_parameter("mph", [128, MCOLS], u8, isOutput=False)
    mpc_d = nc.declare_dram_parameter("mpc", [128, MCOLS], u8, isOutput=False)
    y_d = nc.declare_dram_parameter("y", [BS, 1], f32, isOutput=True)

    SLOTS = 8  # step-slots per bank round (bank = 512 f32 = 8 x 64 cols)

    with ExitStack() as ctx:
        tc = ctx.enter_context(tile.TileContext(nc))
        const = ctx.enter_context(tc.tile_pool(name="const", bufs=1))
        xpool = ctx.enter_context(tc.tile_pool(name="xtp", bufs=1))
        psum0 = ctx.enter_context(
            tc.tile_pool(name="g0", bufs=1, space=bass.MemorySpace.PSUM)
        )
        psum1 = ctx.enter_context(
            tc.tile_pool(name="g1", bufs=1, space=bass.MemorySpace.PSUM)
        )
        spool = ctx.enter_context(tc.tile_pool(name="sig", bufs=2))
        mpool = ctx.enter_context(tc.tile_pool(name="masks", bufs=1))
        wpool = ctx.enter_context(tc.tile_pool(name="work", bufs=2))
        hpool = ctx.enter_context(tc.tile_pool(name="state", bufs=2))

        cst = const.tile([128, C_COLS], bf16)
        nc.sync.dma_start(cst[:], cst_d[:])
        xt = xpool.tile([I + 1, T * BS], bf16)
        nc.sync.dma_start(xt[:], xt_d[:])
        mph = mpool.tile([128, MCOLS], u8, tag="mph")
        nc.sync.dma_start(mph[:], mph_d[:])
        mpc = mpool.tile([128, MCOLS], u8, tag="mpc")
        nc.sync.dma_start(mpc[:], mpc_d[:])

        bias0 = cst[:, C_BIAS : C_BIAS + 1]

        # ACT preamble: absorb the cst DMA tick on ACT and preload the
        # sigmoid/tanh table set before the timestep loop.
        scratch = const.tile([128, 1], bf16)
        nc.scalar.activation(scratch[:], cst[:, 0:1], AF.Copy)
        nc.scalar.activation(scratch[:], cst[:, 0:1], AF.Sigmoid, bias=bias0)
        nc.scalar.activation(scratch[:], cst[:, 0:1], AF.Tanh, bias=bias0)
        # DVE preamble: absorb the two mask DMA ticks so in-loop mask reads
        # never add a second wait on top of same-engine pipeline waits.
        scr8 = const.tile([1, 2], u8)
        nc.vector.tensor_copy(scr8[0:1, 0:1], mph[0:1, 0:1])
        nc.vector.tensor_copy(scr8[0:1, 1:2], mpc[0:1, 0:1])

        # ---- initial state (DVE so consumers' waits stay single-source) ----
        h0_prev = hpool.tile([H0, BS], bf16, tag="h0")
        nc.vector.memset(h0_prev[:], 0.0)
        h1_slot0 = hpool.tile([H1 + 1, BS], bf16, tag="h1")
        nc.vector.memset(h1_slot0[H1 : H1 + 1, :], 1.0)
        h1_prev = hpool.tile([H1 + 1, BS], bf16, tag="h1")
        nc.vector.memset(h1_prev[0:H1, :], 0.0)
        nc.vector.memset(h1_prev[H1 : H1 + 1, :], 1.0)
        c0_prev = wpool.tile([128, BS], f32, tag="c0")
        nc.vector.memset(c0_prev[:], 0.0)
        c1_prev = wpool.tile([128, BS], f32, tag="c1")
        nc.vector.memset(c1_prev[:], 0.0)

        # two long-lived 4-bank PSUM tiles; slot regions recycle mod SLOTS
        TL0 = psum0.tile([128, 4 * 512], f32, tag="TL0")
        TL1 = psum1.tile([128, 4 * 512], f32, tag="TL1")
        # [128, gate, col] views (gate-major banks)
        T0v = TL0[:].rearrange("p (g c) -> p g c", g=4)
        T1v = TL1[:].rearrange("p (g c) -> p g c", g=4)

        h0_new = h0_prev
        for u in range(n_steps + 1):
            s0 = u % SLOTS            # L0 slot for step u
            s1 = (u - 1) % SLOTS      # L1 slot for step u-1

            # ---- PE: x-part batch for the next 8 L0 steps (chain shadow) ----
            if u % SLOTS == 0 and u < n_steps:
                for j in range(4):
                    nc.tensor.matmul(
                        T0v[:, j, :],
                        cst[0 : I + 1, C_WIH0 + j * 128 : C_WIH0 + (j + 1) * 128],
                        xt[:, u * BS : (u + SLOTS) * BS],
                        start=True,
                        stop=False,
                    )
            # ---- PE: L0 recurrent (the only matmuls on the chain) ----
            if u < n_steps:
                for j in range(4):
                    nc.tensor.matmul(
                        T0v[:, j, s0 * BS : (s0 + 1) * BS],
                        cst[0:H0, C_WHH0 + j * 128 : C_WHH0 + (j + 1) * 128],
                        h0_prev[:],
                        start=False,
                        stop=True,
                    )
            # ---- PE: L1 recurrent for step u-1 (ready: h1_{u-2} done) ----
            if u >= 1:
                for j in range(4):
                    nc.tensor.matmul(
                        T1v[0:H1, j, s1 * BS : (s1 + 1) * BS],
                        cst[0 : H1 + 1, C_WHH1 + j * H1 : C_WHH1 + (j + 1) * H1],
                        h1_prev[:],
                        start=False,
                        stop=True,
                    )

            # ---- ACT: sigmoids (L0 then L1), then tanhs ----
            S0 = spool.tile([128, 256], bf16, tag="S0")
            if u < n_steps:
                nc.scalar.activation(
                    S0[:].rearrange("p (g c) -> p g c", g=4),
                    T0v[:, :, s0 * BS : (s0 + 1) * BS],
                    AF.Sigmoid,
                    bias=bias0,
                )
            S1 = spool.tile([128, 256], bf16, tag="S1")
            if u >= 1:
                nc.scalar.activation(
                    S1[:].rearrange("p (g c) -> p g c", g=4),
                    T1v[:, :, s1 * BS : (s1 + 1) * BS],
                    AF.Sigmoid,
                    bias=bias0,
                )

            Mh = mph[:, u * 128 : (u + 1) * 128]
            Mc = mpc[:, u * 128 : (u + 1) * 128]

            # ---- DVE: L0 chain ops ----
            if u < n_steps:
                Pp0 = wpool.tile([128, BS], bf16, tag="Pp0")
                nc.vector.scalar_tensor_tensor(
                    Pp0[:], S0[:, 192:256], 0.5, S0[:, 0:64], ALU.subtract, ALU.mult
                )
                v0 = wpool.tile([128, BS], f32, tag="v0")
                nc.vector.tensor_tensor(v0[:], S0[:, 64:128], c0_prev[:], ALU.mult)
                vc0 = wpool.tile([128, BS], f32, tag="vc0")
                nc.vector.scalar_tensor_tensor(
                    vc0[:], Pp0[:], 2.0, v0[:], ALU.mult, ALU.add
                )
                o20 = wpool.tile([128, BS], bf16, tag="o20")
                nc.vector.scalar_tensor_tensor(
                    o20[:], S0[:, 128:192], MASK_SCALE, Mh[:, 0:64], ALU.mult, ALU.mult
                )
                T0 = wpool.tile([128, BS], bf16, tag="T0")
                nc.scalar.activation(T0[:], vc0[:], AF.Tanh, bias=bias0)

            # ---- DVE: L1 ops (decoupled, lag 1) ----
            if u >= 1:
                Pp1 = wpool.tile([128, BS], bf16, tag="Pp1")
                nc.vector.scalar_tensor_tensor(
                    Pp1[:], S1[:, 192:256], 0.5, S1[:, 0:64], ALU.subtract, ALU.mult
                )
                v1 = wpool.tile([128, BS], f32, tag="v1")
                nc.vector.tensor_tensor(v1[:], S1[:, 64:128], c1_prev[:], ALU.mult)
                vc1 = wpool.tile([128, BS], f32, tag="vc1")
                nc.vector.scalar_tensor_tensor(
                    vc1[:], Pp1[:], 2.0, v1[:], ALU.mult, ALU.add
                )
                o21 = wpool.tile([128, BS], bf16, tag="o21")
                nc.vector.scalar_tensor_tensor(
                    o21[:], S1[:, 128:192], MASK_SCALE, Mh[:, 64:128], ALU.mult, ALU.mult
                )
                T1 = wpool.tile([128, BS], bf16, tag="T1")
                nc.scalar.activation(T1[:], vc1[:], AF.Tanh, bias=bias0)

            # ---- h updates (chain: h0 right after tanh0) ----
            if u < n_steps:
                h0_new = hpool.tile([H0, BS], bf16, tag="h0")
                nc.vector.tensor_tensor(h0_new[:], o20[:], T0[:], ALU.mult)
            if u >= 1:
                h1_new = hpool.tile([H1 + 1, BS], bf16, tag="h1")
                nc.vector.tensor_tensor(
                    h1_new[0:H1, :], o21[0:H1, :], T1[0:H1, :], ALU.mult
                )
                h1_prev = h1_new

            # ---- c carries (off-chain until next step's v) ----
            if u < n_steps:
                c0_new = wpool.tile([128, BS], f32, tag="c0")
                nc.vector.scalar_tensor_tensor(
                    c0_new[:], vc0[:], MASK_SCALE, Mc[:, 0:64], ALU.mult, ALU.mult
                )
                c0_prev = c0_new
            if u >= 1:
                c1_new = wpool.tile([128, BS], f32, tag="c1")
                nc.vector.scalar_tensor_tensor(
                    c1_new[:], vc1[:], MASK_SCALE, Mc[:, 64:128], ALU.mult, ALU.mult
                )
                c1_prev = c1_new

            # ---- PE: L1 x-part for step u (needs h0_u; runs at chain end) ----
            if u < n_steps:
                for j in range(4):
                    nc.tensor.matmul(
                        T1v[0:H1, j, s0 * BS : (s0 + 1) * BS],
                        cst[0:H0, C_WIH1 + j * H1 : C_WIH1 + (j + 1) * H1],
                        h0_new[:],
                        start=(s0 == 0),
                        stop=False,
                    )
                h0_prev = h0_new

        # ---- final projection: y = h1_255 @ W_lin.T + b_lin  -> [64, 1] ----
        yp = T0v[0:BS, 0, 0:1]
        nc.tensor.matmul(
            yp, h1_prev[:], cst[0 : H1 + 1, C_WLIN : C_WLIN + 1], start=True, stop=True
        )
        y_sb = const.tile([BS, 1], f32)
        nc.scalar.copy(y_sb[:], yp)
        nc.sync.dma_start(y_d[:], y_sb[:])

    _split_multiwaits(nc)
    return nc


def _split_multiwaits(nc):
    """This toolchain's walrus accepts at most one semaphore wait per
    instruction. Split any extra waits onto standalone EventSemaphore
    instructions inserted just before the offending instruction on the same
    engine queue (in-order execution preserves semantics exactly)."""
    from concourse import mybir

    n = 0
    for fn in nc.m.functions:
        for bb in fn.blocks:
            out = []
            for inst in bb.instructions:
                si = inst.sync_info
                if si is not None and si.on_wait and len(si.on_wait) > 1:
                    waits = list(si.on_wait)
                    for w in waits[:-1]:
                        n += 1
                        out.append(
                            mybir.InstEventSemaphore(
                                name=f"I-wsplit-{n}",
                                engine=inst.engine,
                                ins=[],
                                outs=[],
                                sync_info=mybir.SyncInfo(on_wait=[w], on_update=[]),
                            )
                        )
                    inst.sync_info = mybir.SyncInfo(
                        on_wait=[waits[-1]], on_update=list(si.on_update)
                    )
                out.append(inst)
            bb.instructions = out


def _prep_consts(W_ih0, W_hh0, b_ih0, b_hh0, W_ih1, W_hh1, b_ih1, b_hh1, W_lin, b_lin):
    f = np.float32
    import ml_dtypes

    b0 = (b_ih0 + b_hh0).astype(f)
    b1 = (b_ih1 + b_hh1).astype(f)
    cst = np.zeros((128, C_COLS), f)
    for j, g in enumerate(GATE_ORDER):
        m = 2.0 if g == G_GATE else 1.0
        cst[0:I, C_WIH0 + j * H0 : C_WIH0 + (j + 1) * H0] = (
            m * W_ih0[g * H0 : (g + 1) * H0].T
        )
        cst[I, C_WIH0 + j * H0 : C_WIH0 + (j + 1) * H0] = m * b0[g * H0 : (g + 1) * H0]
        cst[0:H0, C_WHH0 + j * H0 : C_WHH0 + (j + 1) * H0] = (
            m * W_hh0[g * H0 : (g + 1) * H0].T
        )
        cst[0:H0, C_WIH1 + j * H1 : C_WIH1 + (j + 1) * H1] = (
            m * W_ih1[g * H1 : (g + 1) * H1].T
        )
        cst[0:H1, C_WHH1 + j * H1 : C_WHH1 + (j + 1) * H1] = (
            m * W_hh1[g * H1 : (g + 1) * H1].T
        )
        cst[H1, C_WHH1 + j * H1 : C_WHH1 + (j + 1) * H1] = m * b1[g * H1 : (g + 1) * H1]
    cst[0:H1, C_WLIN] = W_lin[0]
    cst[H1, C_WLIN] = b_lin[0]
    return cst.astype(ml_dtypes.bfloat16)


def kernel(
    input_seq,
    mask_h0,
    mask_c0,
    mask_h1,
    mask_c1,
    W_ih0,
    W_hh0,
    b_ih0,
    b_hh0,
    W_ih1,
    W_hh1,
    b_ih1,
    b_hh1,
    W_lin,
    b_lin,
):
    import sys

    for p in ("/opt/trn_rl_repo",):
        if p not in sys.path:
            sys.path.insert(0, p)
    from concourse.bass_utils import run_bass_kernel_spmd

    f = np.float32
    import ml_dtypes

    input_seq = np.asarray(input_seq, f)
    mask_h0 = np.asarray(mask_h0, f)
    mask_c0 = np.asarray(mask_c0, f)
    mask_h1 = np.asarray(mask_h1, f)
    mask_c1 = np.asarray(mask_c1, f)
    args = [np.asarray(a, f) for a in (W_ih0, W_hh0, b_ih0, b_hh0,
                                       W_ih1, W_hh1, b_ih1, b_hh1, W_lin, b_lin)]
    cst = _prep_consts(*args)

    in_maps = []
    for c in range(NCORES):
        lo, hi = c * BS, (c + 1) * BS
        xs = input_seq[lo:hi]  # [BS, T, I]
        xt = np.empty((I + 1, T * BS), ml_dtypes.bfloat16)
        xt[0:I] = xs.transpose(2, 1, 0).reshape(I, T * BS)
        xt[I] = 1.0
        u8 = np.uint8
        mph3 = np.zeros((T + 1, 128, 128), u8)
        mpc3 = np.zeros((T + 1, 128, 128), u8)
        mph3[0:T, :, 0:64] = (mask_h0[:, lo:hi, :] != 0).transpose(0, 2, 1)
        mph3[1 : T + 1, 0:H1, 64:128] = (mask_h1[:, lo:hi, :] != 0).transpose(0, 2, 1)
        mpc3[0:T, :, 0:64] = (mask_c0[:, lo:hi, :] != 0).transpose(0, 2, 1)
        mpc3[1 : T + 1, 0:H1, 64:128] = (mask_c1[:, lo:hi, :] != 0).transpose(0, 2, 1)
        mph = np.ascontiguousarray(mph3.transpose(1, 0, 2).reshape(128, -1))
        mpc = np.ascontiguousarray(mpc3.transpose(1, 0, 2).reshape(128, -1))
        in_maps.append({"cst": cst, "xt": xt, "mph": mph, "mpc": mpc})

    if "nc" not in _CACHED:
        _CACHED["nc"] = _build_program()
    _CACHED["in_maps"] = in_maps
    res = run_bass_kernel_spmd(_CACHED["nc"], in_maps, list(range(NCORES)))
    out = np.concatenate([res.results[c]["y"] for c in range(NCORES)], axis=0)
    return out.astype(f)


# revision 19
# speedup vs baseline: 3.1398x; 1.0018x over previous
"""Two-layer dropout-masked LSTM (B=512, T=256, I=64, H0=128, H1=32) on 8 trn2 cores.

Data-parallel over batch: 64 rows/core. State kept transposed [feature, batch],
all matmul operands bf16 (PSUM accumulates f32); c-state math stays f32.

PSUM layout: two long-lived 4-bank tiles, bank = gate (order i,f,o,g).
  TL0 (layer 0): bank j holds 8 step-slots of 64 cols; the x+bias part of 8
  steps is batched into ONE N=512 matmul per gate per round (start=True also
  clears the bank's has_written bits for the round); the per-step recurrent
  matmul (start=False) accumulates into its 64-col slot.
  TL1 (layer 1): bank j holds 8 step-slots; per step one x-part matmul
  (vs h0_t) and one recurrent matmul (vs h1_{t-1}, bias via ones row).

L0 and L1 run as two decoupled pipelines (L1 lags by one step) with separate
sigmoid calls and separate elementwise sets, so only L0's 4 recurrent matmuls
+ sigmoid + P'/v/vc + tanh + h0 sit on the serial recurrence chain; L1's
matmuls and ops execute in the chain's engine-idle shadow.

The toolchain's walrus build supports at most ONE semaphore wait per
instruction: _split_multiwaits() moves extra waits onto standalone
EventSemaphore instructions. An ACT preamble absorbs the const-DMA tick and
preloads the sigmoid/tanh table set; a DVE preamble absorbs the mask DMA
ticks. tanh(g) = 2*sigmoid(2g) - 1 with the 2x pre-scaled into the g-gate
weights.
"""

import numpy as np

B, T, I, H0, H1 = 512, 256, 64, 128, 32
NCORES = 8
BS = B // NCORES  # 64
# col-block order within the per-gate banks: i, f, o, g (pytorch rows i,f,g,o)
GATE_ORDER = [0, 1, 3, 2]
G_GATE = 2  # pytorch block index of the tanh gate, pre-scaled by 2

# packed-constant tensor column layout
C_WIH0 = 0        # rows 0:65,  cols 0:512
C_WHH0 = 512      # rows 0:128, cols 512:1024
C_WIH1 = 1024     # rows 0:128, cols 1024:1152
C_WHH1 = 1152     # rows 0:33,  cols 1152:1280
C_WLIN = 1280     # rows 0:33,  col 1280
C_BIAS = 1281     # zeros col
C_COLS = 1282
MASK_SCALE = float(np.float32(1.0) / np.float32(1.0 - 0.4))

_CACHED = {}


def _build_program(n_steps=T):
    import concourse.bass as bass
    import concourse.tile as tile
    from concourse import mybir
    from contextlib import ExitStack

    f32 = mybir.dt.float32
    bf16 = mybir.dt.bfloat16
    u8 = mybir.dt.uint8
    AF = mybir.ActivationFunctionType
    ALU = mybir.AluOpType

    nc = bass.Bass()

    MCOLS = (T + 1) * 128
    cst_d = nc.declare_dram_parameter("cst", [128, C_COLS], bf16, isOutput=False)
    xt_d = nc.declare_dram_parameter("xt", [I + 1, T * BS], bf16, isOutput=False)
    mph_d = nc.declare_dram_parameter("mph", [128, MCOLS], u8, isOutput=False)
    mpc_d = nc.declare_dram_parameter("mpc", [128, MCOLS], u8, isOutput=False)
    y_d = nc.declare_dram_parameter("y", [BS, 1], f32, isOutput=True)

    SLOTS = 8  # step-slots per bank round (bank = 512 f32 = 8 x 64 cols)

    with ExitStack() as ctx:
        tc = ctx.enter_context(tile.TileContext(nc))
        const = ctx.enter_context(tc.tile_pool(name="const", bufs=1))
        xpool = ctx.enter_context(tc.tile_pool(name="xtp", bufs=1))
        psum0 = ctx.enter_context(
            tc.tile_pool(name="g0", bufs=1, space=bass.MemorySpace.PSUM)
        )
        psum1 = ctx.enter_context(
            tc.tile_pool(name="g1", bufs=1, space=bass.MemorySpace.PSUM)
        )
        spool = ctx.enter_context(tc.tile_pool(name="sig", bufs=2))
        mpool = ctx.enter_context(tc.tile_pool(name="masks", bufs=1))
        wpool = ctx.enter_context(tc.tile_pool(name="work", bufs=2))
        hpool = ctx.enter_context(tc.tile_pool(name="state", bufs=2))

        cst = const.tile([128, C_COLS], bf16)
        nc.sync.dma_start(cst[:], cst_d[:])
        xt = xpool.tile([I + 1, T * BS], bf16)
        nc.sync.dma_start(xt[:], xt_d[:])
        mph = mpool.tile([128, MCOLS], u8, tag="mph")
        nc.sync.dma_start(mph[:], mph_d[:])
        mpc = mpool.tile([128, MCOLS], u8, tag="mpc")
        nc.sync.dma_start(mpc[:], mpc_d[:])

        bias0 = cst[:, C_BIAS : C_BIAS + 1]

        # ACT preamble: absorb the cst DMA tick on ACT and preload the
        # sigmoid/tanh table set before the timestep loop.
        scratch = const.tile([128, 1], bf16)
        nc.scalar.activation(scratch[:], cst[:, 0:1], AF.Copy)
        nc.scalar.activation(scratch[:], cst[:, 0:1], AF.Sigmoid, bias=bias0)
        nc.scalar.activation(scratch[:], cst[:, 0:1], AF.Tanh, bias=bias0)
        # DVE preamble: absorb the two mask DMA ticks so in-loop mask reads
        # never add a second wait on top of same-engine pipeline waits.
        scr8 = const.tile([1, 2], u8)
        nc.vector.tensor_copy(scr8[0:1, 0:1], mph[0:1, 0:1])
        nc.vector.tensor_copy(scr8[0:1, 1:2], mpc[0:1, 0:1])

        # ---- initial state (DVE so consumers' waits stay single-source) ----
        h0_prev = hpool.tile([H0, BS], bf16, tag="h0")
        nc.vector.memset(h0_prev[:], 0.0)
        h1_slot0 = hpool.tile([H1 + 1, BS], bf16, tag="h1")
        nc.vector.memset(h1_slot0[H1 : H1 + 1, :], 1.0)
        h1_prev = hpool.tile([H1 + 1, BS], bf16, tag="h1")
        nc.vector.memset(h1_prev[0:H1, :], 0.0)
        nc.vector.memset(h1_prev[H1 : H1 + 1, :], 1.0)
        c0_prev = wpool.tile([128, BS], f32, tag="c0")
        nc.vector.memset(c0_prev[:], 0.0)
        c1_prev = wpool.tile([128, BS], f32, tag="c1")
        nc.vector.memset(c1_prev[:], 0.0)

        # two long-lived 4-bank PSUM tiles; slot regions recycle mod SLOTS
        TL0 = psum0.tile([128, 4 * 512], f32, tag="TL0")
        TL1 = psum1.tile([128, 4 * 512], f32, tag="TL1")
        # [128, gate, col] views (gate-major banks)
        T0v = TL0[:].rearrange("p (g c) -> p g c", g=4)
        T1v = TL1[:].rearrange("p (g c) -> p g c", g=4)

        # pre-loop: x-part for steps 0-3 (opens each TL0 bank's first round)
        for j in range(4):
            nc.tensor.matmul(
                T0v[:, j, 0 : 4 * BS],
                cst[0 : I + 1, C_WIH0 + j * 128 : C_WIH0 + (j + 1) * 128],
                xt[:, 0 : 4 * BS],
                start=True,
                stop=False,
            )

        h0_new = h0_prev
        for u in range(n_steps + 1):
            s0 = u % SLOTS            # L0 slot for step u
            s1 = (u - 1) % SLOTS      # L1 slot for step u-1

            # ---- PE: L0 recurrent (the only matmuls on the chain) ----
            if u < n_steps:
                for j in range(4):
                    nc.tensor.matmul(
                        T0v[:, j, s0 * BS : (s0 + 1) * BS],
                        cst[0:H0, C_WHH0 + j * 128 : C_WHH0 + (j + 1) * 128],
                        h0_prev[:],
                        start=False,
                        stop=True,
                    )
            # ---- PE: L1 x-part for step u-1 (off-chain: after rec in queue,
            # before L1rec so the slot's accumulation order is x then rec;
            # the s1==0 instance opens TL1's round / clears its bits) ----
            if u >= 1:
                for j in range(4):
                    nc.tensor.matmul(
                        T1v[0:H1, j, s1 * BS : (s1 + 1) * BS],
                        cst[0:H0, C_WIH1 + j * H1 : C_WIH1 + (j + 1) * H1],
                        h0_prev[:],
                        start=(s1 == 0),
                        stop=False,
                    )
            # ---- PE: L1 recurrent for step u-1 (ready: h1_{u-2} done) ----
            if u >= 1:
                for j in range(4):
                    nc.tensor.matmul(
                        T1v[0:H1, j, s1 * BS : (s1 + 1) * BS],
                        cst[0 : H1 + 1, C_WHH1 + j * H1 : C_WHH1 + (j + 1) * H1],
                        h1_prev[:],
                        start=False,
                        stop=True,
                    )

            # ---- ACT: sigmoids (L0 then L1), then tanhs ----
            S0 = spool.tile([128, 256], bf16, tag="S0")
            if u < n_steps:
                nc.scalar.activation(
                    S0[:].rearrange("p (g c) -> p g c", g=4),
                    T0v[:, :, s0 * BS : (s0 + 1) * BS],
                    AF.Sigmoid,
                    bias=bias0,
                )

            # ---- PE: x-part batch for steps u+1..u+4 (issued after this
            # step's mms; a zero-valued [1,1] anchor matmul reads S0 so the
            # PE cannot touch the TL0 banks until sig_L0(u) has retired —
            # PE-write + ACT-read of the same PSUM bank is a fatal collision.
            # The slot-0 half opens the round: anchor clears bank 0's
            # has_written bits, xbatch j>=1 clears banks 1-3. ----
            if u % 4 == 3 and u + 1 < n_steps:
                sf_ = (u + 1) % SLOTS
                nx = min(4, n_steps - (u + 1))
                nc.tensor.matmul(
                    T0v[0:1, 0, sf_ * BS : sf_ * BS + 1],
                    cst[0:1, C_BIAS : C_BIAS + 1],
                    S0[0:1, 0:1],
                    start=(sf_ == 0),
                    stop=False,
                )
                for j in range(4):
                    nc.tensor.matmul(
                        T0v[:, j, sf_ * BS : (sf_ + nx) * BS],
                        cst[0 : I + 1, C_WIH0 + j * 128 : C_WIH0 + (j + 1) * 128],
                        xt[:, (u + 1) * BS : (u + 1 + nx) * BS],
                        start=(sf_ == 0 and j >= 1),
                        stop=False,
                    )
            S1 = spool.tile([128, 256], bf16, tag="S1")
            if u >= 1:
                nc.scalar.activation(
                    S1[:].rearrange("p (g c) -> p g c", g=4),
                    T1v[:, :, s1 * BS : (s1 + 1) * BS],
                    AF.Sigmoid,
                    bias=bias0,
                )

            Mh = mph[:, u * 128 : (u + 1) * 128]
            Mc = mpc[:, u * 128 : (u + 1) * 128]

            # ---- DVE: L0 chain ops, h0 immediately after tanh0 ----
            if u < n_steps:
                Pp0 = wpool.tile([128, BS], bf16, tag="Pp0")
                nc.vector.scalar_tensor_tensor(
                    Pp0[:], S0[:, 192:256], 0.5, S0[:, 0:64], ALU.subtract, ALU.mult
                )
                o20 = wpool.tile([128, BS], bf16, tag="o20")
                nc.vector.scalar_tensor_tensor(
                    o20[:], S0[:, 128:192], MASK_SCALE, Mh[:, 0:64], ALU.mult, ALU.mult
                )
                v0 = wpool.tile([128, BS], f32, tag="v0")
                nc.vector.tensor_tensor(v0[:], S0[:, 64:128], c0_prev[:], ALU.mult)
                vc0 = wpool.tile([128, BS], f32, tag="vc0")
                nc.vector.scalar_tensor_tensor(
                    vc0[:], Pp0[:], 2.0, v0[:], ALU.mult, ALU.add
                )
                T0 = wpool.tile([128, BS], bf16, tag="T0")
                nc.scalar.activation(T0[:], vc0[:], AF.Tanh, bias=bias0)
                h0_new = hpool.tile([H0, BS], bf16, tag="h0")
                nc.vector.tensor_tensor(h0_new[:], o20[:], T0[:], ALU.mult)

            # ---- DVE: L1 ops (decoupled, lag 1) ----
            if u >= 1:
                Pp1 = wpool.tile([128, BS], bf16, tag="Pp1")
                nc.vector.scalar_tensor_tensor(
                    Pp1[:], S1[:, 192:256], 0.5, S1[:, 0:64], ALU.subtract, ALU.mult
                )
                o21 = wpool.tile([128, BS], bf16, tag="o21")
                nc.vector.scalar_tensor_tensor(
                    o21[:], S1[:, 128:192], MASK_SCALE, Mh[:, 64:128], ALU.mult, ALU.mult
                )
                v1 = wpool.tile([128, BS], f32, tag="v1")
                nc.vector.tensor_tensor(v1[:], S1[:, 64:128], c1_prev[:], ALU.mult)
                vc1 = wpool.tile([128, BS], f32, tag="vc1")
                nc.vector.scalar_tensor_tensor(
                    vc1[:], Pp1[:], 2.0, v1[:], ALU.mult, ALU.add
                )
                T1 = wpool.tile([128, BS], bf16, tag="T1")
                nc.scalar.activation(T1[:], vc1[:], AF.Tanh, bias=bias0)
                h1_new = hpool.tile([H1 + 1, BS], bf16, tag="h1")
                nc.vector.tensor_tensor(
                    h1_new[0:H1, :], o21[0:H1, :], T1[0:H1, :], ALU.mult
                )
                h1_prev = h1_new

            # ---- c carries (off-chain until next step's v) ----
            if u < n_steps:
                c0_new = wpool.tile([128, BS], f32, tag="c0")
                nc.vector.scalar_tensor_tensor(
                    c0_new[:], vc0[:], MASK_SCALE, Mc[:, 0:64], ALU.mult, ALU.mult
                )
                c0_prev = c0_new
            if u >= 1:
                c1_new = wpool.tile([128, BS], f32, tag="c1")
                nc.vector.scalar_tensor_tensor(
                    c1_new[:], vc1[:], MASK_SCALE, Mc[:, 64:128], ALU.mult, ALU.mult
                )
                c1_prev = c1_new

            if u < n_steps:
                h0_prev = h0_new

        # ---- final projection: y = h1_255 @ W_lin.T + b_lin  -> [64, 1] ----
        yp = T0v[0:BS, 0, 0:1]
        nc.tensor.matmul(
            yp, h1_prev[:], cst[0 : H1 + 1, C_WLIN : C_WLIN + 1], start=True, stop=True
        )
        y_sb = const.tile([BS, 1], f32)
        nc.scalar.copy(y_sb[:], yp)
        nc.sync.dma_start(y_d[:], y_sb[:])

    _split_multiwaits(nc)
    return nc


def _split_multiwaits(nc):
    """This toolchain's walrus accepts at most one semaphore wait per
    instruction. Split any extra waits onto standalone EventSemaphore
    instructions inserted just before the offending instruction on the same
    engine queue (in-order execution preserves semantics exactly)."""
    from concourse import mybir

    n = 0
    for fn in nc.m.functions:
        for bb in fn.blocks:
            out = []
            for inst in bb.instructions:
                si = inst.sync_info
                if si is not None and si.on_wait and len(si.on_wait) > 1:
                    waits = list(si.on_wait)
                    for w in waits[:-1]:
                        n += 1
                        out.append(
                            mybir.InstEventSemaphore(
                                name=f"I-wsplit-{n}",
                                engine=inst.engine,
                                ins=[],
                                outs=[],
                                sync_info=mybir.SyncInfo(on_wait=[w], on_update=[]),
                            )
                        )
                    inst.sync_info = mybir.SyncInfo(
                        on_wait=[waits[-1]], on_update=list(si.on_update)
                    )
                out.append(inst)
            bb.instructions = out


def _prep_consts(W_ih0, W_hh0, b_ih0, b_hh0, W_ih1, W_hh1, b_ih1, b_hh1, W_lin, b_lin):
    f = np.float32
    import ml_dtypes

    b0 = (b_ih0 + b_hh0).astype(f)
    b1 = (b_ih1 + b_hh1).astype(f)
    cst = np.zeros((128, C_COLS), f)
    for j, g in enumerate(GATE_ORDER):
        m = 2.0 if g == G_GATE else 1.0
        cst[0:I, C_WIH0 + j * H0 : C_WIH0 + (j + 1) * H0] = (
            m * W_ih0[g * H0 : (g + 1) * H0].T
        )
        cst[I, C_WIH0 + j * H0 : C_WIH0 + (j + 1) * H0] = m * b0[g * H0 : (g + 1) * H0]
        cst[0:H0, C_WHH0 + j * H0 : C_WHH0 + (j + 1) * H0] = (
            m * W_hh0[g * H0 : (g + 1) * H0].T
        )
        cst[0:H0, C_WIH1 + j * H1 : C_WIH1 + (j + 1) * H1] = (
            m * W_ih1[g * H1 : (g + 1) * H1].T
        )
        cst[0:H1, C_WHH1 + j * H1 : C_WHH1 + (j + 1) * H1] = (
            m * W_hh1[g * H1 : (g + 1) * H1].T
        )
        cst[H1, C_WHH1 + j * H1 : C_WHH1 + (j + 1) * H1] = m * b1[g * H1 : (g + 1) * H1]
    cst[0:H1, C_WLIN] = W_lin[0]
    cst[H1, C_WLIN] = b_lin[0]
    return cst.astype(ml_dtypes.bfloat16)


def kernel(
    input_seq,
    mask_h0,
    mask_c0,
    mask_h1,
    mask_c1,
    W_ih0,
    W_hh0,
    b_ih0,
    b_hh0,
    W_ih1,
    W_hh1,
    b_ih1,
    b_hh1,
    W_lin,
    b_lin,
):
    import sys

    for p in ("/opt/trn_rl_repo",):
        if p not in sys.path:
            sys.path.insert(0, p)
    from concourse.bass_utils import run_bass_kernel_spmd

    f = np.float32
    import ml_dtypes

    input_seq = np.asarray(input_seq, f)
    mask_h0 = np.asarray(mask_h0, f)
    mask_c0 = np.asarray(mask_c0, f)
    mask_h1 = np.asarray(mask_h1, f)
    mask_c1 = np.asarray(mask_c1, f)
    args = [np.asarray(a, f) for a in (W_ih0, W_hh0, b_ih0, b_hh0,
                                       W_ih1, W_hh1, b_ih1, b_hh1, W_lin, b_lin)]
    cst = _prep_consts(*args)

    in_maps = []
    for c in range(NCORES):
        lo, hi = c * BS, (c + 1) * BS
        xs = input_seq[lo:hi]  # [BS, T, I]
        xt = np.empty((I + 1, T * BS), ml_dtypes.bfloat16)
        xt[0:I] = xs.transpose(2, 1, 0).reshape(I, T * BS)
        xt[I] = 1.0
        u8 = np.uint8
        mph3 = np.zeros((T + 1, 128, 128), u8)
        mpc3 = np.zeros((T + 1, 128, 128), u8)
        mph3[0:T, :, 0:64] = (mask_h0[:, lo:hi, :] != 0).transpose(0, 2, 1)
        mph3[1 : T + 1, 0:H1, 64:128] = (mask_h1[:, lo:hi, :] != 0).transpose(0, 2, 1)
        mpc3[0:T, :, 0:64] = (mask_c0[:, lo:hi, :] != 0).transpose(0, 2, 1)
        mpc3[1 : T + 1, 0:H1, 64:128] = (mask_c1[:, lo:hi, :] != 0).transpose(0, 2, 1)
        mph = np.ascontiguousarray(mph3.transpose(1, 0, 2).reshape(128, -1))
        mpc = np.ascontiguousarray(mpc3.transpose(1, 0, 2).reshape(128, -1))
        in_maps.append({"cst": cst, "xt": xt, "mph": mph, "mpc": mpc})

    if "nc" not in _CACHED:
        _CACHED["nc"] = _build_program()
    _CACHED["in_maps"] = in_maps
    res = run_bass_kernel_spmd(_CACHED["nc"], in_maps, list(range(NCORES)))
    out = np.concatenate([res.results[c]["y"] for c in range(NCORES)], axis=0)
    return out.astype(f)
